# revision 41
# baseline (speedup 1.0000x reference)
"""Single-head causal attention with RoPE on 8 Trainium2 NeuronCores.

Problem: x:(8,2048,1024), Wq/Wk/Wv:(1024,64) -> out:(8,2048,64)
  q = rope(x@Wq); k = rope(x@Wk); v = x@Wv
  out = softmax(causal(q k^T / sqrt(64))) @ v

Sharding: data-parallel over batch B=8, one batch element per core.

v9 (trace-driven evolution of the 64us v2 baseline):
  - DMA was the wall: effective HBM read bw was ~200GB/s because the xT
    pieces were 2KB-per-partition strided across 4MB of DRAM. Host packs
    xT as [chunk, half, 128, 4, 512]: each piece one contiguous 512KB
    burst (few pieces - each completion receipt costs ~2us).
  - RoPE tables: only 32 unique sin + 32 cos rows exist; a [64, 2048]
    bf16 table (256KB, first on the wire) is expanded on-chip to the
    full [128, 2048] tables by PE selector matmuls; the ramp-time
    evictions ride the otherwise-idle ACT engine.
  - Per-chunk pipeline with a cost-budgeted filler queue: A(i) qk-proj,
    rope pieces (psum cast / R2 rot matmul / cos-sin muls / half-swap
    matmul), V(i), b(i) interleave into earlier attention phases at
    ~0.5us granularity; emission order keeps PE instructions from
    waiting on DVE results queued behind them (no head-of-line stalls).
  - The k'/q' partition-swapped copy (for the row-paired score streams)
    is ONE PE permutation matmul per chunk; an SBUF-SBUF DMA here has
    ~5us completion latency under HBM load and gated the scores.
  - Host-side normalization: kernel emits [num(64);den(1)] x 512 bf16
    per chunk; host divides. Kills 16 PE transposes + recip/mul and the
    out-rearrange DMA.
  - A 12-matmul junk bridge spans the DMA ramp so the HAM clock-gate
    reaches K=8/8 before the real projections (v2 lost ~13us to
    half-clock windows); the drain-time junk heartbeat measured WORSE
    (crowds out phase-C scores) and is off by default.
  - b() transpose halves go to SEPARATE psum banks (concurrent row-group
    transpose streams into one bank = HW write-port collision -> NC
    error).
"""

import collections
import os
import numpy as np
import ml_dtypes

B, T, C, H = 8, 2048, 1024, 64
NCORES = 8
CHUNK = 512
NCHUNK = T // CHUNK  # 4
NSB = T // 128       # 16 s-blocks
NCB = C // 128       # 8 c-blocks
VHALF = CHUNK // 2   # 256

bf16 = ml_dtypes.bfloat16

JUNK_ON = os.environ.get("K_JUNK", "0") == "1"
MAXPHASE = int(os.environ.get("K_MAXPHASE", "3"))

SHUF_MASK = [i ^ 1 for i in range(32)]
PI = float(np.pi)
TWO_PI = float(2 * np.pi)


# ---------------------------------------------------------------- host consts
def _build_consts():
    half = H // 2
    inv_freq = (1.0 / (10000.0 ** (np.arange(half, dtype=np.float32) / half))).astype(
        np.float32
    )
    # unique-row rope table: 32 sin rows + 32 cos rows (the full [128, T]
    # tables repeat each frequency row 4x with signs); expanded on-chip by
    # a PE selector matmul. sctab rows 0:32 = sin(t*f_j), 32:64 = cos.
    t = np.arange(T, dtype=np.float64)
    ang = t[None, :] * inv_freq.astype(np.float64)[:, None]  # (32, T)
    sctab = np.concatenate([np.sin(ang), np.cos(ang)], axis=0).astype(bf16)

    # selectors [64, 2, 128]: out_row h of table k = sum_j sel[j, k, h] *
    # sctab[j]; sin rows get the rope rotation sign (-1 on even rows),
    # cos rows are plain repeats
    sel = np.zeros((64, 2, 128), np.float32)
    for h in range(128):
        hp = h % 64
        j = hp // 2
        sel[j, 0, h] = 1.0          # sinsin
        sel[32 + j, 1, h] = 1.0     # coscos
    sel = sel.astype(bf16)

    sl = np.arange(128)
    trimask = (sl[:, None] <= sl[None, :]).astype(bf16)  # (128, 128)

    identb2 = np.concatenate([np.eye(H), np.eye(H)], axis=0).astype(bf16)  # (128, 64)

    # rot matrix (PE): rot = R2.T @ qk with rot[2i] = -qk[2i+1], rot[2i+1] = qk[2i]
    Rm = np.zeros((H, H), np.float32)
    for i in range(H // 2):
        Rm[2 * i, 2 * i + 1] = -1.0
        Rm[2 * i + 1, 2 * i] = 1.0
    r2 = np.zeros((128, 128), np.float32)
    r2[0:H, 0:H] = Rm.T
    r2[H:128, H:128] = Rm.T
    r2 = r2.astype(bf16)

    # half-swap permutation: swapk = swapm.T @ qkrope puts k' at rows 0:64
    # and q' at rows 64:128 (out[m] = qkrope[(m+64)%128])
    swapm = np.zeros((128, 128), np.float32)
    swapm[(np.arange(128) + H) % 128, np.arange(128)] = 1.0
    swapm = swapm.astype(bf16)

    return sctab, sel, trimask, identb2, swapm, r2


# ---------------------------------------------------------------- bass program
def _build_bass():
    import concourse.mybir as mybir
    import concourse.tile as tile
    from concourse import bacc
    from concourse.bass import ts

    BF = mybir.dt.bfloat16
    F32 = mybir.dt.float32
    Exp = mybir.ActivationFunctionType.Exp
    Sin = mybir.ActivationFunctionType.Sin
    Mod = mybir.AluOpType.mod
    Mult = mybir.AluOpType.mult
    Add = mybir.AluOpType.add
    Min = mybir.AluOpType.min

    nc = bacc.Bacc(
        "TRN2",
        target_bir_lowering=False,
        debug=False,
        enable_asserts=False,
        num_devices=NCORES,
    )

    # xT prepacked on host: [chunk, half, 128, 4, 512] -> each (chunk,
    # half) DMA piece is a fully contiguous 512KB burst (few pieces: DMA
    # completion receipts cost ~2us each and pace the ramp)
    xT_d = nc.dram_tensor(
        "xTp", [NCHUNK, 2, 128, NCB // 2, CHUNK], BF, kind="ExternalInput"
    )
    wqk_d = nc.dram_tensor("wqkp", [128, NCB, 128], BF, kind="ExternalInput")
    wv_d = nc.dram_tensor("wvp", [128, NCB, H], BF, kind="ExternalInput")
    sctab_d = nc.dram_tensor("sctab", [H, T], BF, kind="ExternalInput")
    sel_d = nc.dram_tensor("sel", [H, 2, 128], BF, kind="ExternalInput")
    trimask_d = nc.dram_tensor("trimask", [128, 128], BF, kind="ExternalInput")
    identb_d = nc.dram_tensor("identb2", [128, H], BF, kind="ExternalInput")
    swapm_d = nc.dram_tensor("swapm", [128, 128], BF, kind="ExternalInput")
    r2_d = nc.dram_tensor("r2", [128, 128], BF, kind="ExternalInput")
    # raw [num(64); den(1)] per chunk; host divides
    out_d = nc.dram_tensor("out", [NCHUNK, H + 1, CHUNK], BF, kind="ExternalOutput")

    with tile.TileContext(nc) as tc:
        with (
            tc.tile_pool(name="persist", bufs=1) as persist,
            tc.tile_pool(name="work", bufs=3) as work,
            tc.tile_pool(name="pexpp", bufs=5) as pexpp,
            tc.tile_pool(name="ps_scratch", bufs=2, space="PSUM") as ps_scratch,
            tc.tile_pool(name="ps_sc", bufs=2, space="PSUM") as ps_sc,
            tc.tile_pool(name="ps_out", bufs=2, space="PSUM") as ps_out,
        ):
            # ---- persistent SBUF tensors
            wqk_sb = persist.tile([128, NCB, 128], BF)
            wv_sb = persist.tile([128, NCB, H], BF)
            coscos_sb = persist.tile([128, T], BF)
            sinsin_sb = persist.tile([128, T], BF)
            trimask_sb = persist.tile([128, 128], BF)
            identb_sb = persist.tile([128, H], BF)
            swapm_sb = persist.tile([128, 128], BF)
            r2_sb = persist.tile([128, 128], BF)
            sctab_sb = persist.tile([H, T], BF)
            sel_sb = persist.tile([H, 2, 128], BF)
            xT_sb = persist.tile([128, NCHUNK, NCB, CHUNK], BF)
            qkrope = persist.tile([128, T], BF)   # q' rows 0:64, k' rows 64:128
            swapk = persist.tile([128, T], BF)    # k' rows 0:64, q' rows 64:128
            vT_sb = persist.tile([128, NCHUNK, VHALF], BF)  # per-chunk col-halves
            vnat = persist.tile([128, NSB, H + 1], BF)

            # ---- sync HWDGE queue: ordered by need time; the rope table
            # first (its psum eviction gates chunk 0's projection slot)
            nc.sync.dma_start(out=sctab_sb[:], in_=sctab_d.ap())
            nc.sync.dma_start(out=sel_sb[:], in_=sel_d.ap())
            nc.sync.dma_start(out=wqk_sb[:], in_=wqk_d.ap())
            for i in range(NCHUNK):
                for h in range(2):
                    nc.sync.dma_start(
                        out=xT_sb[:, i, 4 * h : 4 * h + 4], in_=xT_d.ap()[i, h]
                    )
                if i == 0:
                    nc.sync.dma_start(out=wv_sb[:], in_=wv_d.ap())
            # ---- gpsimd (SWDGE): small consts on the other ring
            nc.gpsimd.dma_start(out=identb_sb[:], in_=identb_d.ap())
            nc.gpsimd.dma_start(out=swapm_sb[:], in_=swapm_d.ap())
            nc.gpsimd.dma_start(out=r2_sb[:], in_=r2_d.ap())
            nc.gpsimd.dma_start(out=trimask_sb[:], in_=trimask_d.ap())

            # PE warmup: junk matmuls so the HAM clock-gate starts opening
            # while the first DMAs land
            zwarm = persist.tile([128, CHUNK], BF)
            nc.vector.memset(zwarm[:], 0.0)
            nc.vector.memset(vnat[:], 1.0)  # ones col (64); cols 0:64 overwritten
            warm_ps = ps_sc.tile([128, 2, CHUNK], F32, tag="sc", name="warm")
            for w in range(3):
                nc.tensor.matmul(
                    warm_ps[:, 0, :],
                    zwarm[:, 0:128],
                    zwarm[:],
                    start=(w == 0),
                    stop=(w == 2),
                )

            # ---- expand the [64, T] unique-row table to the full
            # [128, T] sinsin/coscos tables via PE selector matmuls (exact
            # bf16 passthrough); per chunk so chunk 0's rope isn't queued
            # behind later chunks' table-eviction copies on the DVE
            def emit_extab(i, on_act=False):
                tsl0 = ts(i, CHUNK)
                for tab_sb, k in ((sinsin_sb, 0), (coscos_sb, 1)):
                    e_ps = ps_scratch.tile(
                        [128, CHUNK], F32, tag="scr", name=f"ex{i}_{k}"
                    )
                    nc.tensor.matmul(
                        e_ps[:], sel_sb[:, k, :], sctab_sb[:, tsl0],
                        start=True, stop=True,
                    )
                    if on_act:
                        # ACT is idle during the ramp; keeps the eviction off
                        # the DVE critical path and un-gates the scr slot
                        nc.scalar.copy(out=tab_sb[:, tsl0], in_=e_ps[:])
                    else:
                        nc.vector.tensor_copy(out=tab_sb[:, tsl0], in_=e_ps[:])

            # ---------------- emission units
            qk_tiles = {}

            def emit_qk_alloc(i):
                qk_tiles[i] = ps_scratch.tile(
                    [128, CHUNK], F32, tag="scr", name=f"qk{i}"
                )

            def emit_qk(i, c2):
                # 2 c-blocks of the qk projection accumulation for chunk i
                qk_ps = qk_tiles[i]
                for c in (c2, c2 + 1):
                    nc.tensor.matmul(
                        qk_ps[:],
                        wqk_sb[:, c, :],
                        xT_sb[:, i, c, :],
                        start=(c == 0),
                        stop=(c == NCB - 1),
                        skip_group_check=True,
                    )

            qkS_tiles = {}
            rot_tiles = {}

            def emit_cast(i):
                qkS = work.tile([128, CHUNK], BF, tag="qkS", name=f"qkS{i}")
                qkS_tiles[i] = qkS
                nc.vector.tensor_copy(out=qkS[:], in_=qk_tiles[i][:])

            def emit_rot(i):
                rot_ps = ps_scratch.tile(
                    [128, CHUNK], F32, tag="scr", name=f"rot{i}"
                )
                rot_tiles[i] = rot_ps
                nc.tensor.matmul(
                    rot_ps[:], r2_sb[:], qkS_tiles[i][:], start=True, stop=True
                )

            def emit_ropemul(i):
                tsl = ts(i, CHUNK)
                tmp1 = work.tile([128, CHUNK], BF, tag="tmp1", name=f"t1_{i}")
                nc.vector.tensor_mul(tmp1[:], qkS_tiles[i][:], coscos_sb[:, tsl])
                tmp2 = work.tile([128, CHUNK], BF, tag="tmp2", name=f"t2_{i}")
                nc.vector.tensor_mul(tmp2[:], rot_tiles[i][:], sinsin_sb[:, tsl])
                nc.vector.tensor_add(qkrope[:, tsl], tmp1[:], tmp2[:])

            def emit_swap(i):
                # half-swapped copy for the two row-tiled score streams via a
                # PE permutation matmul (an SBUF-SBUF DMA here has ~5us
                # completion latency under HBM load and gates the scores)
                tsl = ts(i, CHUNK)
                swap_ps = ps_scratch.tile(
                    [128, CHUNK], F32, tag="scr", name=f"swp{i}"
                )
                nc.tensor.matmul(
                    swap_ps[:], swapm_sb[:], qkrope[:, tsl], start=True, stop=True
                )
                nc.vector.tensor_copy(out=swapk[:, tsl], in_=swap_ps[:])

            v_tiles = {}

            def emit_v_alloc(i):
                v_tiles[i] = ps_scratch.tile(
                    [128, VHALF], F32, tag="scr", name=f"v{i}"
                )

            def emit_v(i, c2):
                # 2 c-blocks of chunk i's v-projection, col-tiled over the
                # chunk's two 256-halves so both PE column groups stream
                # concurrently
                v_ps = v_tiles[i]
                for c in (c2, c2 + 1):
                    nc.tensor.matmul(
                        v_ps[0:H, :],
                        wv_sb[:, c, :],
                        xT_sb[:, i, c, 0:VHALF],
                        start=(c == 0),
                        stop=(c == NCB - 1),
                        skip_group_check=True,
                    )
                    nc.tensor.matmul(
                        v_ps[H:128, :],
                        wv_sb[:, c, :],
                        xT_sb[:, i, c, VHALF:CHUNK],
                        start=(c == 0),
                        stop=(c == NCB - 1),
                        skip_group_check=True,
                    )
                if c2 == NCB - 2:
                    nc.vector.tensor_copy(out=vT_sb[:, i, :], in_=v_ps[:])

            def emit_b(i):
                # transpose chunk i's 4 s-blocks to natural layout; one psum
                # tile per row-group half: the two transpose streams run
                # concurrently and must drain into DIFFERENT banks
                for half_ in range(2):
                    vn_ps = ps_scratch.tile(
                        [128, 2, H], BF, tag="scr", name=f"vn{i}_{half_}"
                    )
                    base = H * half_
                    for j in range(2):
                        nc.tensor.transpose(
                            vn_ps[:, j, :],
                            vT_sb[base : base + H, i, ts(j, 128)],
                            identb_sb[base : base + H, :],
                        )
                    first = 4 * i + 2 * half_
                    nc.vector.tensor_copy(
                        out=vnat[:, first : first + 2, 0:H], in_=vn_ps[:]
                    )

            out_tiles = {}

            def emit_evict(i):
                out_ps = out_tiles[i]
                outS = work.tile([H + 1, CHUNK], BF, tag="outS", name=f"oS{i}")
                nc.vector.tensor_copy(out=outS[:], in_=out_ps[:])
                nc.gpsimd.dma_start(out=out_d.ap()[i], in_=outS[:])

            # ---------------- filler queue (cost_ns, key, fn)
            fillers = collections.deque()
            done_keys = set()
            junk_budget = [24 if JUNK_ON else 0]

            def drain_one():
                cost, key, fn = fillers.popleft()
                fn()
                if key is not None:
                    done_keys.add(key)
                return cost

            def drain(budget):
                while fillers and budget > 0:
                    budget -= drain_one()
                # fillers dry: keep the PE dense so HAM holds K=8/8
                while budget > 0 and junk_budget[0] > 0:
                    junk_ps = ps_scratch.tile(
                        [128, CHUNK], F32, tag="scr",
                        name=f"junk{junk_budget[0]}",
                    )
                    nc.tensor.matmul(
                        junk_ps[:], zwarm[:, 0:128], zwarm[:],
                        start=True, stop=True, skip_group_check=True,
                    )
                    junk_budget[0] -= 1
                    budget -= 216

            def drain_until(key):
                if key in done_keys:
                    return
                while fillers:
                    cost, k, fn = fillers[0]
                    drain_one()
                    if k == key:
                        return

            def queue_A(i):
                def qk_unit(c2, first):
                    def fn():
                        if first:
                            emit_qk_alloc(i)
                        emit_qk(i, c2)
                    return fn

                for c2 in range(0, NCB, 2):
                    fillers.append((432, None, qk_unit(c2, c2 == 0)))
                fillers.append((100, None, lambda: emit_cast(i)))
                fillers.append((216, None, lambda: emit_rot(i)))
                fillers.append((100, None, lambda: emit_ropemul(i)))
                fillers.append((300, ("rotrope", i), lambda: emit_swap(i)))

            def queue_V(i):
                def v_unit(c2, first):
                    def fn():
                        if first:
                            emit_v_alloc(i)
                        emit_v(i, c2)
                    return fn

                for c2 in range(0, NCB, 2):
                    fillers.append((440, None, v_unit(c2, c2 == 0)))
                fillers.append((300, ("b", i), lambda: emit_b(i)))

            # ---------------- phase C
            def phase_c(i, diag_pos=None, budget=500):
                drain_until(("rotrope", i))
                bkey = ("b", i)
                out_ps = ps_out.tile([H + 1, CHUNK], F32, tag="out", name=f"o{i}")
                out_tiles[i] = out_ps
                started = [False]

                punits = [("pair", p) for p in range(2 * i)]
                dunits = [("diag", 0), ("diag", 1)]
                if diag_pos is None:
                    diag_pos = len(punits)
                units = punits[:diag_pos] + dunits + punits[diag_pos:]
                staged = []

                def emit_scores(u):
                    kind, idx = u
                    sc2 = ps_sc.tile(
                        [128, 2, CHUNK], F32, tag="sc", name=f"s{i}_{kind}{idx}"
                    )
                    if kind == "pair":
                        sb = 2 * idx
                        nc.tensor.matmul(
                            sc2[:, 0, :],
                            swapk[0:H, ts(sb, 128)],
                            qkrope[0:H, ts(i, CHUNK)],
                            start=True,
                            stop=True,
                        )
                        nc.tensor.matmul(
                            sc2[:, 1, :],
                            qkrope[H:128, ts(sb + 1, 128)],
                            swapk[H:128, ts(i, CHUNK)],
                            start=True,
                            stop=True,
                        )
                    else:
                        j0 = 2 * idx
                        lo0 = 128 * j0
                        nc.tensor.matmul(
                            sc2[:, 0, lo0:CHUNK],
                            swapk[0:H, ts(4 * i + j0, 128)],
                            qkrope[0:H, i * CHUNK + lo0 : (i + 1) * CHUNK],
                            start=True,
                            stop=True,
                        )
                        # stream B starts at lo0 too: the extra cols ride in
                        # stream A's concurrency shadow and initialize the
                        # region the merged exp reads
                        nc.tensor.matmul(
                            sc2[:, 1, lo0:CHUNK],
                            qkrope[H:128, ts(4 * i + j0 + 1, 128)],
                            swapk[H:128, i * CHUNK + lo0 : (i + 1) * CHUNK],
                            start=True,
                            stop=True,
                        )
                    staged.append((kind, idx, sc2))

                def emit_exp_num(stage, last_unit):
                    kind, idx, sc2 = stage
                    pexp2 = pexpp.tile(
                        [128, 2, CHUNK], BF, tag="pexp", name=f"p{i}_{kind}{idx}"
                    )
                    if kind == "pair":
                        nc.scalar.activation(
                            out=pexp2[:], in_=sc2[:], func=Exp, scale=0.125
                        )
                        for h_ in range(2):
                            sb = 2 * idx + h_
                            st = not started[0]
                            started[0] = True
                            nc.tensor.matmul(
                                out_ps[:],
                                vnat[:, sb, :],
                                pexp2[:, h_, :],
                                start=st,
                                stop=(last_unit and h_ == 1),
                                skip_group_check=True,
                            )
                    else:
                        j0 = 2 * idx
                        lo0 = 128 * j0
                        nc.scalar.activation(
                            out=pexp2[:, :, lo0:CHUNK],
                            in_=sc2[:, :, lo0:CHUNK],
                            func=Exp,
                            scale=0.125,
                        )
                        # trimask corners on DVE (off PE critical path)
                        for h_ in range(2):
                            lo = 128 * (j0 + h_)
                            nc.vector.tensor_mul(
                                pexp2[:, h_, lo : lo + 128],
                                pexp2[:, h_, lo : lo + 128],
                                trimask_sb[:],
                            )
                        halves = [0, 1]
                        if not started[0]:
                            # a start=True matmul resets the WHOLE psum tile,
                            # so the chunk's first AV must be one full-width
                            # start=True instruction (corner already masked)
                            assert j0 == 0
                            started[0] = True
                            nc.tensor.matmul(
                                out_ps[:],
                                vnat[:, 4 * i, :],
                                pexp2[:, 0, :],
                                start=True,
                                stop=False,
                                skip_group_check=True,
                            )
                            halves = [1]
                        # unmasked AV tails fire straight after exp
                        for h_ in halves:
                            sb = 4 * i + j0 + h_
                            lo = 128 * (j0 + h_)
                            if lo + 128 < CHUNK:
                                nc.tensor.matmul(
                                    out_ps[:, lo + 128 : CHUNK],
                                    vnat[:, sb, :],
                                    pexp2[:, h_, lo + 128 : CHUNK],
                                    start=False,
                                    stop=False,
                                    skip_group_check=True,
                                )
                        # masked corners after the DVE multiplies
                        for h_ in halves:
                            sb = 4 * i + j0 + h_
                            lo = 128 * (j0 + h_)
                            nc.tensor.matmul(
                                out_ps[:, lo : lo + 128],
                                vnat[:, sb, :],
                                pexp2[:, h_, lo : lo + 128],
                                start=False,
                                stop=(last_unit and h_ == 1),
                                skip_group_check=True,
                            )

                first_diag = diag_pos
                for n, u in enumerate(units):
                    if n == first_diag:
                        drain_until(bkey)
                    emit_scores(u)
                    if n > 0:
                        emit_exp_num(staged.pop(0), False)
                    drain(budget)
                emit_exp_num(staged.pop(0), True)

            # ---------------- top-level schedule: per-chunk pipeline with
            # A/V/b of later chunks as fillers inside earlier C phases
            # junk bridge: keep the PE busy until the rope table + x-stream
            # land (a >3.4us idle window re-throttles HAM)
            for w in range(12):
                nc.tensor.matmul(
                    warm_ps[:, 1, :], zwarm[:, 0:128], zwarm[:],
                    start=(w == 0), stop=(w == 11), skip_group_check=True,
                )
            emit_extab(0, on_act=True)
            # chunk-0 chain, ordered so no PE instruction waits on a DVE
            # result that is produced behind it in the queue
            emit_qk_alloc(0)
            for c2 in range(0, NCB, 2):
                emit_qk(0, c2)
            emit_cast(0)
            emit_v_alloc(0)
            for c2 in range(0, NCB, 2):
                emit_v(0, c2)
            emit_rot(0)
            emit_ropemul(0)
            emit_b(0)
            done_keys.add(("b", 0))
            emit_swap(0)
            done_keys.add(("rotrope", 0))
            emit_extab(1, on_act=True)
            # A(1) qk matmuls directly: they fill the PE idle window while
            # the chunk-0 rope chain runs on the DVE (the scores that follow
            # them in the queue wait on that chain anyway)
            emit_qk_alloc(1)
            for c2 in range(0, NCB, 2):
                emit_qk(1, c2)

            fillers.append((100, None, lambda: emit_cast(1)))
            fillers.append((216, None, lambda: emit_rot(1)))
            fillers.append((100, None, lambda: emit_ropemul(1)))
            fillers.append((300, ("rotrope", 1), lambda: emit_swap(1)))
            queue_V(1)
            fillers.append((450, None, lambda: emit_extab(2)))
            queue_A(2)
            queue_V(2)
            fillers.append((450, None, lambda: emit_extab(3)))
            queue_A(3)
            queue_V(3)

            phase_c(0, budget=3000)
            emit_evict(0)
            if MAXPHASE >= 1:
                phase_c(1, budget=2200)
                emit_evict(1)
            if MAXPHASE >= 2:
                phase_c(2, budget=1100)
                emit_evict(2)
            if MAXPHASE >= 3:
                phase_c(3, diag_pos=2, budget=500)
                emit_evict(3)

    nc.compile()
    return nc


_NC_CACHE = None


def _get_nc():
    global _NC_CACHE
    if _NC_CACHE is None:
        _NC_CACHE = _build_bass()
    return _NC_CACHE


def make_in_maps(x, Wq, Wk, Wv):
    """Host-side prep: shard over batch + precompute constants."""
    sctab, sel, trimask, identb2, swapm, r2 = _build_consts()
    wqk = np.concatenate([Wq, Wk], axis=1).astype(bf16)  # (C, 128)
    wv = Wv.astype(bf16)
    wqkp = np.ascontiguousarray(wqk.reshape(NCB, 128, 128).transpose(1, 0, 2))
    wvp = np.ascontiguousarray(wv.reshape(NCB, 128, H).transpose(1, 0, 2))
    in_maps = []
    for b in range(B):
        xT = x[b].T.astype(bf16)  # (C, T)
        # [chunk, half, 128, 4, 512]: contiguous 512KB per (chunk, half)
        xTp = np.ascontiguousarray(
            xT.reshape(2, 4, 128, NCHUNK, CHUNK).transpose(3, 0, 2, 1, 4)
        )
        in_maps.append(
            {
                "xTp": xTp,
                "wqkp": wqkp,
                "wvp": wvp,
                "sctab": sctab,
                "sel": sel,
                "trimask": trimask,
                "identb2": identb2,
                "swapm": swapm,
                "r2": r2,
            }
        )
    return in_maps


def postprocess(raw):
    """[NCHUNK, 65, CHUNK] num/den -> (T, H) normalized output."""
    raw = np.asarray(raw, dtype=np.float32)
    num = raw[:, 0:H, :]                  # (4, 64, 512)
    den = raw[:, H, :]                    # (4, 512)
    out = num / den[:, None, :]
    return np.ascontiguousarray(out.transpose(0, 2, 1).reshape(T, H))


def kernel(x, Wq, Wk, Wv):
    from concourse.bass_utils import run_bass_kernel_spmd

    x = np.asarray(x, dtype=np.float32)
    Wq = np.asarray(Wq, dtype=np.float32)
    Wk = np.asarray(Wk, dtype=np.float32)
    Wv = np.asarray(Wv, dtype=np.float32)

    nc = _get_nc()
    in_maps = make_in_maps(x, Wq, Wk, Wv)
    res = run_bass_kernel_spmd(nc, in_maps, core_ids=list(range(NCORES)))
    out = np.stack([postprocess(r["out"]) for r in res.results])  # (B, T, H)
    return np.ascontiguousarray(out.astype(np.float32))


# revision 42
# speedup vs baseline: 1.0057x; 1.0057x over previous
"""Single-head causal attention with RoPE on 8 Trainium2 NeuronCores.

Problem: x:(8,2048,1024), Wq/Wk/Wv:(1024,64) -> out:(8,2048,64)
  q = rope(x@Wq); k = rope(x@Wk); v = x@Wv
  out = softmax(causal(q k^T / sqrt(64))) @ v

Sharding: data-parallel over batch B=8, one batch element per core.

v9 (trace-driven evolution of the 64us v2 baseline):
  - DMA was the wall: effective HBM read bw was ~200GB/s because the xT
    pieces were 2KB-per-partition strided across 4MB of DRAM. Host packs
    xT as [chunk, half, 128, 4, 512]: each piece one contiguous 512KB
    burst (few pieces - each completion receipt costs ~2us).
  - RoPE tables: only 32 unique sin + 32 cos rows exist; a [64, 2048]
    bf16 table (256KB, first on the wire) is expanded on-chip to the
    full [128, 2048] tables by PE selector matmuls; the ramp-time
    evictions ride the otherwise-idle ACT engine.
  - Per-chunk pipeline with a cost-budgeted filler queue: A(i) qk-proj,
    rope pieces (psum cast / R2 rot matmul / cos-sin muls / half-swap
    matmul), V(i), b(i) interleave into earlier attention phases at
    ~0.5us granularity; emission order keeps PE instructions from
    waiting on DVE results queued behind them (no head-of-line stalls).
  - The k'/q' partition-swapped copy (for the row-paired score streams)
    is ONE PE permutation matmul per chunk; an SBUF-SBUF DMA here has
    ~5us completion latency under HBM load and gated the scores.
  - Host-side normalization: kernel emits [num(64);den(1)] x 512 bf16
    per chunk; host divides. Kills 16 PE transposes + recip/mul and the
    out-rearrange DMA.
  - A 12-matmul junk bridge spans the DMA ramp so the HAM clock-gate
    reaches K=8/8 before the real projections (v2 lost ~13us to
    half-clock windows); the drain-time junk heartbeat measured WORSE
    (crowds out phase-C scores) and is off by default.
  - b() transpose halves go to SEPARATE psum banks (concurrent row-group
    transpose streams into one bank = HW write-port collision -> NC
    error).
"""

import collections
import os
import numpy as np
import ml_dtypes

B, T, C, H = 8, 2048, 1024, 64
NCORES = 8
CHUNK = 512
NCHUNK = T // CHUNK  # 4
NSB = T // 128       # 16 s-blocks
NCB = C // 128       # 8 c-blocks
VHALF = CHUNK // 2   # 256

bf16 = ml_dtypes.bfloat16

JUNK_ON = os.environ.get("K_JUNK", "0") == "1"
MAXPHASE = int(os.environ.get("K_MAXPHASE", "3"))

SHUF_MASK = [i ^ 1 for i in range(32)]
PI = float(np.pi)
TWO_PI = float(2 * np.pi)


# ---------------------------------------------------------------- host consts
def _build_consts():
    half = H // 2
    inv_freq = (1.0 / (10000.0 ** (np.arange(half, dtype=np.float32) / half))).astype(
        np.float32
    )
    # unique-row rope table: 32 sin rows + 32 cos rows (the full [128, T]
    # tables repeat each frequency row 4x with signs); expanded on-chip by
    # a PE selector matmul. sctab rows 0:32 = sin(t*f_j), 32:64 = cos.
    t = np.arange(T, dtype=np.float64)
    ang = t[None, :] * inv_freq.astype(np.float64)[:, None]  # (32, T)
    sctab = np.concatenate([np.sin(ang), np.cos(ang)], axis=0).astype(bf16)

    # selectors [64, 2, 128]: out_row h of table k = sum_j sel[j, k, h] *
    # sctab[j]; sin rows get the rope rotation sign (-1 on even rows),
    # cos rows are plain repeats
    sel = np.zeros((64, 2, 128), np.float32)
    for h in range(128):
        hp = h % 64
        j = hp // 2
        sel[j, 0, h] = 1.0          # sinsin
        sel[32 + j, 1, h] = 1.0     # coscos
    sel = sel.astype(bf16)

    sl = np.arange(128)
    trimask = (sl[:, None] <= sl[None, :]).astype(bf16)  # (128, 128)

    identb2 = np.concatenate([np.eye(H), np.eye(H)], axis=0).astype(bf16)  # (128, 64)

    # rot matrix (PE): rot = R2.T @ qk with rot[2i] = -qk[2i+1], rot[2i+1] = qk[2i]
    Rm = np.zeros((H, H), np.float32)
    for i in range(H // 2):
        Rm[2 * i, 2 * i + 1] = -1.0
        Rm[2 * i + 1, 2 * i] = 1.0
    r2 = np.zeros((128, 128), np.float32)
    r2[0:H, 0:H] = Rm.T
    r2[H:128, H:128] = Rm.T
    r2 = r2.astype(bf16)

    # half-swap permutation: swapk = swapm.T @ qkrope puts k' at rows 0:64
    # and q' at rows 64:128 (out[m] = qkrope[(m+64)%128])
    swapm = np.zeros((128, 128), np.float32)
    swapm[(np.arange(128) + H) % 128, np.arange(128)] = 1.0
    swapm = swapm.astype(bf16)

    return sctab, sel, trimask, identb2, swapm, r2


# ---------------------------------------------------------------- bass program
def _build_bass():
    import concourse.mybir as mybir
    import concourse.tile as tile
    from concourse import bacc
    from concourse.bass import ts

    BF = mybir.dt.bfloat16
    F32 = mybir.dt.float32
    Exp = mybir.ActivationFunctionType.Exp
    Sin = mybir.ActivationFunctionType.Sin
    Mod = mybir.AluOpType.mod
    Mult = mybir.AluOpType.mult
    Add = mybir.AluOpType.add
    Min = mybir.AluOpType.min

    nc = bacc.Bacc(
        "TRN2",
        target_bir_lowering=False,
        debug=False,
        enable_asserts=False,
        num_devices=NCORES,
    )

    # xT prepacked on host: [chunk, half, 128, 4, 512] -> each (chunk,
    # half) DMA piece is a fully contiguous 512KB burst (few pieces: DMA
    # completion receipts cost ~2us each and pace the ramp)
    xT_d = nc.dram_tensor(
        "xTp", [NCHUNK, 2, 128, NCB // 2, CHUNK], BF, kind="ExternalInput"
    )
    wqk_d = nc.dram_tensor("wqkp", [128, NCB, 128], BF, kind="ExternalInput")
    wv_d = nc.dram_tensor("wvp", [128, NCB, H], BF, kind="ExternalInput")
    sctab_d = nc.dram_tensor("sctab", [H, T], BF, kind="ExternalInput")
    sel_d = nc.dram_tensor("sel", [H, 2, 128], BF, kind="ExternalInput")
    trimask_d = nc.dram_tensor("trimask", [128, 128], BF, kind="ExternalInput")
    identb_d = nc.dram_tensor("identb2", [128, H], BF, kind="ExternalInput")
    swapm_d = nc.dram_tensor("swapm", [128, 128], BF, kind="ExternalInput")
    r2_d = nc.dram_tensor("r2", [128, 128], BF, kind="ExternalInput")
    # raw [num(64); den(1)] per chunk; host divides
    out_d = nc.dram_tensor("out", [NCHUNK, H + 1, CHUNK], BF, kind="ExternalOutput")

    with tile.TileContext(nc) as tc:
        with (
            tc.tile_pool(name="persist", bufs=1) as persist,
            tc.tile_pool(name="work", bufs=3) as work,
            tc.tile_pool(name="pexpp", bufs=5) as pexpp,
            tc.tile_pool(name="ps_scratch", bufs=2, space="PSUM") as ps_scratch,
            tc.tile_pool(name="ps_sc", bufs=2, space="PSUM") as ps_sc,
            tc.tile_pool(name="ps_out", bufs=2, space="PSUM") as ps_out,
        ):
            # ---- persistent SBUF tensors
            wqk_sb = persist.tile([128, NCB, 128], BF)
            wv_sb = persist.tile([128, NCB, H], BF)
            coscos_sb = persist.tile([128, T], BF)
            sinsin_sb = persist.tile([128, T], BF)
            trimask_sb = persist.tile([128, 128], BF)
            identb_sb = persist.tile([128, H], BF)
            swapm_sb = persist.tile([128, 128], BF)
            r2_sb = persist.tile([128, 128], BF)
            sctab_sb = persist.tile([H, T], BF)
            sel_sb = persist.tile([H, 2, 128], BF)
            xT_sb = persist.tile([128, NCHUNK, NCB, CHUNK], BF)
            qkrope = persist.tile([128, T], BF)   # q' rows 0:64, k' rows 64:128
            swapk = persist.tile([128, T], BF)    # k' rows 0:64, q' rows 64:128
            vT_sb = persist.tile([128, NCHUNK, VHALF], BF)  # per-chunk col-halves
            vnat = persist.tile([128, NSB, H + 1], BF)

            # ---- sync HWDGE queue: ordered by need time; the rope table
            # first (its psum eviction gates chunk 0's projection slot)
            nc.sync.dma_start(out=sctab_sb[:], in_=sctab_d.ap())
            nc.sync.dma_start(out=sel_sb[:], in_=sel_d.ap())
            nc.sync.dma_start(out=wqk_sb[:], in_=wqk_d.ap())
            for i in range(NCHUNK):
                for h in range(2):
                    nc.sync.dma_start(
                        out=xT_sb[:, i, 4 * h : 4 * h + 4], in_=xT_d.ap()[i, h]
                    )
                if i == 0:
                    nc.sync.dma_start(out=wv_sb[:], in_=wv_d.ap())
            # ---- gpsimd (SWDGE): small consts on the other ring
            nc.gpsimd.dma_start(out=identb_sb[:], in_=identb_d.ap())
            nc.gpsimd.dma_start(out=swapm_sb[:], in_=swapm_d.ap())
            nc.gpsimd.dma_start(out=r2_sb[:], in_=r2_d.ap())
            nc.gpsimd.dma_start(out=trimask_sb[:], in_=trimask_d.ap())

            # PE warmup: junk matmuls so the HAM clock-gate starts opening
            # while the first DMAs land
            zwarm = persist.tile([128, CHUNK], BF)
            nc.vector.memset(zwarm[:], 0.0)
            nc.vector.memset(vnat[:], 1.0)  # ones col (64); cols 0:64 overwritten
            warm_ps = ps_sc.tile([128, 2, CHUNK], F32, tag="sc", name="warm")
            for w in range(3):
                nc.tensor.matmul(
                    warm_ps[:, 0, :],
                    zwarm[:, 0:128],
                    zwarm[:],
                    start=(w == 0),
                    stop=(w == 2),
                )

            # ---- expand the [64, T] unique-row table to the full
            # [128, T] sinsin/coscos tables via PE selector matmuls (exact
            # bf16 passthrough); per chunk so chunk 0's rope isn't queued
            # behind later chunks' table-eviction copies on the DVE
            def emit_extab(i, on_act=False):
                tsl0 = ts(i, CHUNK)
                for tab_sb, k in ((sinsin_sb, 0), (coscos_sb, 1)):
                    e_ps = ps_scratch.tile(
                        [128, CHUNK], F32, tag="scr", name=f"ex{i}_{k}"
                    )
                    nc.tensor.matmul(
                        e_ps[:], sel_sb[:, k, :], sctab_sb[:, tsl0],
                        start=True, stop=True,
                    )
                    if on_act:
                        # ACT is idle during the ramp; keeps the eviction off
                        # the DVE critical path and un-gates the scr slot
                        nc.scalar.copy(out=tab_sb[:, tsl0], in_=e_ps[:])
                    else:
                        nc.vector.tensor_copy(out=tab_sb[:, tsl0], in_=e_ps[:])

            # ---------------- emission units
            qk_tiles = {}

            def emit_qk_alloc(i):
                qk_tiles[i] = ps_scratch.tile(
                    [128, CHUNK], F32, tag="scr", name=f"qk{i}"
                )

            def emit_qk(i, c2):
                # 2 c-blocks of the qk projection accumulation for chunk i
                qk_ps = qk_tiles[i]
                for c in (c2, c2 + 1):
                    nc.tensor.matmul(
                        qk_ps[:],
                        wqk_sb[:, c, :],
                        xT_sb[:, i, c, :],
                        start=(c == 0),
                        stop=(c == NCB - 1),
                        skip_group_check=True,
                    )

            qkS_tiles = {}
            rot_tiles = {}

            def emit_cast(i):
                qkS = work.tile([128, CHUNK], BF, tag="qkS", name=f"qkS{i}")
                qkS_tiles[i] = qkS
                nc.vector.tensor_copy(out=qkS[:], in_=qk_tiles[i][:])

            def emit_rot(i):
                rot_ps = ps_scratch.tile(
                    [128, CHUNK], F32, tag="scr", name=f"rot{i}"
                )
                rot_tiles[i] = rot_ps
                nc.tensor.matmul(
                    rot_ps[:], r2_sb[:], qkS_tiles[i][:], start=True, stop=True
                )

            def emit_ropemul(i):
                tsl = ts(i, CHUNK)
                tmp1 = work.tile([128, CHUNK], BF, tag="tmp1", name=f"t1_{i}")
                nc.vector.tensor_mul(tmp1[:], qkS_tiles[i][:], coscos_sb[:, tsl])
                tmp2 = work.tile([128, CHUNK], BF, tag="tmp2", name=f"t2_{i}")
                nc.vector.tensor_mul(tmp2[:], rot_tiles[i][:], sinsin_sb[:, tsl])
                nc.vector.tensor_add(qkrope[:, tsl], tmp1[:], tmp2[:])

            def emit_swap(i):
                # half-swapped copy for the two row-tiled score streams via a
                # PE permutation matmul (an SBUF-SBUF DMA here has ~5us
                # completion latency under HBM load and gates the scores)
                tsl = ts(i, CHUNK)
                swap_ps = ps_scratch.tile(
                    [128, CHUNK], F32, tag="scr", name=f"swp{i}"
                )
                nc.tensor.matmul(
                    swap_ps[:], swapm_sb[:], qkrope[:, tsl], start=True, stop=True
                )
                nc.vector.tensor_copy(out=swapk[:, tsl], in_=swap_ps[:])

            v_tiles = {}

            def emit_v_alloc(i):
                v_tiles[i] = ps_scratch.tile(
                    [128, VHALF], F32, tag="scr", name=f"v{i}"
                )

            def emit_v(i, c2):
                # 2 c-blocks of chunk i's v-projection, col-tiled over the
                # chunk's two 256-halves so both PE column groups stream
                # concurrently
                v_ps = v_tiles[i]
                for c in (c2, c2 + 1):
                    nc.tensor.matmul(
                        v_ps[0:H, :],
                        wv_sb[:, c, :],
                        xT_sb[:, i, c, 0:VHALF],
                        start=(c == 0),
                        stop=(c == NCB - 1),
                        skip_group_check=True,
                    )
                    nc.tensor.matmul(
                        v_ps[H:128, :],
                        wv_sb[:, c, :],
                        xT_sb[:, i, c, VHALF:CHUNK],
                        start=(c == 0),
                        stop=(c == NCB - 1),
                        skip_group_check=True,
                    )
                if c2 == NCB - 2:
                    nc.vector.tensor_copy(out=vT_sb[:, i, :], in_=v_ps[:])

            def emit_b(i):
                # transpose chunk i's 4 s-blocks to natural layout; one psum
                # tile per row-group half: the two transpose streams run
                # concurrently and must drain into DIFFERENT banks
                for half_ in range(2):
                    vn_ps = ps_scratch.tile(
                        [128, 2, H], BF, tag="scr", name=f"vn{i}_{half_}"
                    )
                    base = H * half_
                    for j in range(2):
                        nc.tensor.transpose(
                            vn_ps[:, j, :],
                            vT_sb[base : base + H, i, ts(j, 128)],
                            identb_sb[base : base + H, :],
                        )
                    first = 4 * i + 2 * half_
                    nc.vector.tensor_copy(
                        out=vnat[:, first : first + 2, 0:H], in_=vn_ps[:]
                    )

            out_tiles = {}

            def emit_evict(i):
                out_ps = out_tiles[i]
                outS = work.tile([H + 1, CHUNK], BF, tag="outS", name=f"oS{i}")
                nc.vector.tensor_copy(out=outS[:], in_=out_ps[:])
                nc.gpsimd.dma_start(out=out_d.ap()[i], in_=outS[:])

            # ---------------- filler queue (cost_ns, key, fn)
            fillers = collections.deque()
            done_keys = set()
            junk_budget = [24 if JUNK_ON else 0]

            def drain_one():
                cost, key, fn = fillers.popleft()
                fn()
                if key is not None:
                    done_keys.add(key)
                return cost

            def drain(budget):
                while fillers and budget > 0:
                    budget -= drain_one()
                # fillers dry: keep the PE dense so HAM holds K=8/8
                while budget > 0 and junk_budget[0] > 0:
                    junk_ps = ps_scratch.tile(
                        [128, CHUNK], F32, tag="scr",
                        name=f"junk{junk_budget[0]}",
                    )
                    nc.tensor.matmul(
                        junk_ps[:], zwarm[:, 0:128], zwarm[:],
                        start=True, stop=True, skip_group_check=True,
                    )
                    junk_budget[0] -= 1
                    budget -= 216

            def drain_until(key):
                if key in done_keys:
                    return
                while fillers:
                    cost, k, fn = fillers[0]
                    drain_one()
                    if k == key:
                        return

            def queue_A(i):
                def qk_unit(c2, first):
                    def fn():
                        if first:
                            emit_qk_alloc(i)
                        emit_qk(i, c2)
                    return fn

                for c2 in range(0, NCB, 2):
                    fillers.append((432, None, qk_unit(c2, c2 == 0)))
                fillers.append((100, None, lambda: emit_cast(i)))
                fillers.append((216, None, lambda: emit_rot(i)))
                fillers.append((100, None, lambda: emit_ropemul(i)))
                fillers.append((300, ("rotrope", i), lambda: emit_swap(i)))

            def queue_V(i):
                def v_unit(c2, first):
                    def fn():
                        if first:
                            emit_v_alloc(i)
                        emit_v(i, c2)
                    return fn

                for c2 in range(0, NCB, 2):
                    fillers.append((440, None, v_unit(c2, c2 == 0)))
                fillers.append((300, ("b", i), lambda: emit_b(i)))

            # ---------------- phase C
            def phase_c(i, diag_pos=None, budget=500, entry_budget=0):
                drain_until(("rotrope", i))
                # pop the NEXT chunk's projection+rope fillers now: their
                # DVE chain then queues ahead of this phase's trimask/evict
                # work and completes during our exps instead of gating the
                # next phase's first scores
                drain(entry_budget)
                bkey = ("b", i)
                out_ps = ps_out.tile([H + 1, CHUNK], F32, tag="out", name=f"o{i}")
                out_tiles[i] = out_ps
                started = [False]

                punits = [("pair", p) for p in range(2 * i)]
                dunits = [("diag", 0), ("diag", 1)]
                if diag_pos is None:
                    diag_pos = len(punits)
                units = punits[:diag_pos] + dunits + punits[diag_pos:]
                staged = []

                def emit_scores(u):
                    kind, idx = u
                    sc2 = ps_sc.tile(
                        [128, 2, CHUNK], F32, tag="sc", name=f"s{i}_{kind}{idx}"
                    )
                    if kind == "pair":
                        sb = 2 * idx
                        nc.tensor.matmul(
                            sc2[:, 0, :],
                            swapk[0:H, ts(sb, 128)],
                            qkrope[0:H, ts(i, CHUNK)],
                            start=True,
                            stop=True,
                        )
                        nc.tensor.matmul(
                            sc2[:, 1, :],
                            qkrope[H:128, ts(sb + 1, 128)],
                            swapk[H:128, ts(i, CHUNK)],
                            start=True,
                            stop=True,
                        )
                    else:
                        j0 = 2 * idx
                        lo0 = 128 * j0
                        nc.tensor.matmul(
                            sc2[:, 0, lo0:CHUNK],
                            swapk[0:H, ts(4 * i + j0, 128)],
                            qkrope[0:H, i * CHUNK + lo0 : (i + 1) * CHUNK],
                            start=True,
                            stop=True,
                        )
                        # stream B starts at lo0 too: the extra cols ride in
                        # stream A's concurrency shadow and initialize the
                        # region the merged exp reads
                        nc.tensor.matmul(
                            sc2[:, 1, lo0:CHUNK],
                            qkrope[H:128, ts(4 * i + j0 + 1, 128)],
                            swapk[H:128, i * CHUNK + lo0 : (i + 1) * CHUNK],
                            start=True,
                            stop=True,
                        )
                    staged.append((kind, idx, sc2))

                def emit_exp_num(stage, last_unit):
                    kind, idx, sc2 = stage
                    pexp2 = pexpp.tile(
                        [128, 2, CHUNK], BF, tag="pexp", name=f"p{i}_{kind}{idx}"
                    )
                    if kind == "pair":
                        nc.scalar.activation(
                            out=pexp2[:], in_=sc2[:], func=Exp, scale=0.125
                        )
                        for h_ in range(2):
                            sb = 2 * idx + h_
                            st = not started[0]
                            started[0] = True
                            nc.tensor.matmul(
                                out_ps[:],
                                vnat[:, sb, :],
                                pexp2[:, h_, :],
                                start=st,
                                stop=(last_unit and h_ == 1),
                                skip_group_check=True,
                            )
                    else:
                        j0 = 2 * idx
                        lo0 = 128 * j0
                        nc.scalar.activation(
                            out=pexp2[:, :, lo0:CHUNK],
                            in_=sc2[:, :, lo0:CHUNK],
                            func=Exp,
                            scale=0.125,
                        )
                        # trimask corners on DVE (off PE critical path)
                        for h_ in range(2):
                            lo = 128 * (j0 + h_)
                            nc.vector.tensor_mul(
                                pexp2[:, h_, lo : lo + 128],
                                pexp2[:, h_, lo : lo + 128],
                                trimask_sb[:],
                            )
                        halves = [0, 1]
                        if not started[0]:
                            # a start=True matmul resets the WHOLE psum tile,
                            # so the chunk's first AV must be one full-width
                            # start=True instruction (corner already masked)
                            assert j0 == 0
                            started[0] = True
                            nc.tensor.matmul(
                                out_ps[:],
                                vnat[:, 4 * i, :],
                                pexp2[:, 0, :],
                                start=True,
                                stop=False,
                                skip_group_check=True,
                            )
                            halves = [1]
                        # unmasked AV tails fire straight after exp
                        for h_ in halves:
                            sb = 4 * i + j0 + h_
                            lo = 128 * (j0 + h_)
                            if lo + 128 < CHUNK:
                                nc.tensor.matmul(
                                    out_ps[:, lo + 128 : CHUNK],
                                    vnat[:, sb, :],
                                    pexp2[:, h_, lo + 128 : CHUNK],
                                    start=False,
                                    stop=False,
                                    skip_group_check=True,
                                )
                        # masked corners after the DVE multiplies
                        for h_ in halves:
                            sb = 4 * i + j0 + h_
                            lo = 128 * (j0 + h_)
                            nc.tensor.matmul(
                                out_ps[:, lo : lo + 128],
                                vnat[:, sb, :],
                                pexp2[:, h_, lo : lo + 128],
                                start=False,
                                stop=(last_unit and h_ == 1),
                                skip_group_check=True,
                            )

                first_diag = diag_pos
                for n, u in enumerate(units):
                    if n == first_diag:
                        drain_until(bkey)
                    emit_scores(u)
                    if n > 0:
                        emit_exp_num(staged.pop(0), False)
                    drain(budget)
                emit_exp_num(staged.pop(0), True)

            # ---------------- top-level schedule: per-chunk pipeline with
            # A/V/b of later chunks as fillers inside earlier C phases
            # junk bridge: keep the PE busy until the rope table + x-stream
            # land (a >3.4us idle window re-throttles HAM)
            for w in range(12):
                nc.tensor.matmul(
                    warm_ps[:, 1, :], zwarm[:, 0:128], zwarm[:],
                    start=(w == 0), stop=(w == 11), skip_group_check=True,
                )
            emit_extab(0, on_act=True)
            # chunk-0 chain, ordered so no PE instruction waits on a DVE
            # result that is produced behind it in the queue
            emit_qk_alloc(0)
            for c2 in range(0, NCB, 2):
                emit_qk(0, c2)
            emit_cast(0)
            emit_v_alloc(0)
            for c2 in range(0, NCB, 2):
                emit_v(0, c2)
            emit_rot(0)
            emit_ropemul(0)
            emit_b(0)
            done_keys.add(("b", 0))
            emit_swap(0)
            done_keys.add(("rotrope", 0))
            emit_extab(1, on_act=True)
            # A(1) qk matmuls directly: they fill the PE idle window while
            # the chunk-0 rope chain runs on the DVE (the scores that follow
            # them in the queue wait on that chain anyway)
            emit_qk_alloc(1)
            for c2 in range(0, NCB, 2):
                emit_qk(1, c2)

            fillers.append((100, None, lambda: emit_cast(1)))
            fillers.append((216, None, lambda: emit_rot(1)))
            fillers.append((100, None, lambda: emit_ropemul(1)))
            fillers.append((300, ("rotrope", 1), lambda: emit_swap(1)))
            queue_V(1)
            fillers.append((450, None, lambda: emit_extab(2)))
            queue_A(2)
            queue_V(2)
            fillers.append((450, None, lambda: emit_extab(3)))
            queue_A(3)
            queue_V(3)

            phase_c(0, budget=1700)
            emit_evict(0)
            if MAXPHASE >= 1:
                phase_c(1, budget=700, entry_budget=3000)
                emit_evict(1)
            if MAXPHASE >= 2:
                phase_c(2, budget=600, entry_budget=3000)
                emit_evict(2)
            if MAXPHASE >= 3:
                phase_c(3, diag_pos=2, budget=500, entry_budget=1500)
                emit_evict(3)

    nc.compile()
    return nc


_NC_CACHE = None


def _get_nc():
    global _NC_CACHE
    if _NC_CACHE is None:
        _NC_CACHE = _build_bass()
    return _NC_CACHE


def make_in_maps(x, Wq, Wk, Wv):
    """Host-side prep: shard over batch + precompute constants."""
    sctab, sel, trimask, identb2, swapm, r2 = _build_consts()
    wqk = np.concatenate([Wq, Wk], axis=1).astype(bf16)  # (C, 128)
    wv = Wv.astype(bf16)
    wqkp = np.ascontiguousarray(wqk.reshape(NCB, 128, 128).transpose(1, 0, 2))
    wvp = np.ascontiguousarray(wv.reshape(NCB, 128, H).transpose(1, 0, 2))
    in_maps = []
    for b in range(B):
        xT = x[b].T.astype(bf16)  # (C, T)
        # [chunk, half, 128, 4, 512]: contiguous 512KB per (chunk, half)
        xTp = np.ascontiguousarray(
            xT.reshape(2, 4, 128, NCHUNK, CHUNK).transpose(3, 0, 2, 1, 4)
        )
        in_maps.append(
            {
                "xTp": xTp,
                "wqkp": wqkp,
                "wvp": wvp,
                "sctab": sctab,
                "sel": sel,
                "trimask": trimask,
                "identb2": identb2,
                "swapm": swapm,
                "r2": r2,
            }
        )
    return in_maps


def postprocess(raw):
    """[NCHUNK, 65, CHUNK] num/den -> (T, H) normalized output."""
    raw = np.asarray(raw, dtype=np.float32)
    num = raw[:, 0:H, :]                  # (4, 64, 512)
    den = raw[:, H, :]                    # (4, 512)
    out = num / den[:, None, :]
    return np.ascontiguousarray(out.transpose(0, 2, 1).reshape(T, H))


def kernel(x, Wq, Wk, Wv):
    from concourse.bass_utils import run_bass_kernel_spmd

    x = np.asarray(x, dtype=np.float32)
    Wq = np.asarray(Wq, dtype=np.float32)
    Wk = np.asarray(Wk, dtype=np.float32)
    Wv = np.asarray(Wv, dtype=np.float32)

    nc = _get_nc()
    in_maps = make_in_maps(x, Wq, Wk, Wv)
    res = run_bass_kernel_spmd(nc, in_maps, core_ids=list(range(NCORES)))
    out = np.stack([postprocess(r["out"]) for r in res.results])  # (B, T, H)
    return np.ascontiguousarray(out.astype(np.float32))


# revision 43
# speedup vs baseline: 1.0125x; 1.0067x over previous
"""Single-head causal attention with RoPE on 8 Trainium2 NeuronCores.

Problem: x:(8,2048,1024), Wq/Wk/Wv:(1024,64) -> out:(8,2048,64)
  q = rope(x@Wq); k = rope(x@Wk); v = x@Wv
  out = softmax(causal(q k^T / sqrt(64))) @ v

Sharding: data-parallel over batch B=8, one batch element per core.

v9 (trace-driven evolution of the 64us v2 baseline):
  - DMA was the wall: effective HBM read bw was ~200GB/s because the xT
    pieces were 2KB-per-partition strided across 4MB of DRAM. Host packs
    xT as [chunk, half, 128, 4, 512]: each piece one contiguous 512KB
    burst (few pieces - each completion receipt costs ~2us).
  - RoPE tables: only 32 unique sin + 32 cos rows exist; a [64, 2048]
    bf16 table (256KB, first on the wire) is expanded on-chip to the
    full [128, 2048] tables by PE selector matmuls; the ramp-time
    evictions ride the otherwise-idle ACT engine.
  - Per-chunk pipeline with a cost-budgeted filler queue: A(i) qk-proj,
    rope pieces (psum cast / R2 rot matmul / cos-sin muls / half-swap
    matmul), V(i), b(i) interleave into earlier attention phases at
    ~0.5us granularity; emission order keeps PE instructions from
    waiting on DVE results queued behind them (no head-of-line stalls).
  - The k'/q' partition-swapped copy (for the row-paired score streams)
    is ONE PE permutation matmul per chunk; an SBUF-SBUF DMA here has
    ~5us completion latency under HBM load and gated the scores.
  - Host-side normalization: kernel emits [num(64);den(1)] x 512 bf16
    per chunk; host divides. Kills 16 PE transposes + recip/mul and the
    out-rearrange DMA.
  - A 12-matmul junk bridge spans the DMA ramp so the HAM clock-gate
    reaches K=8/8 before the real projections (v2 lost ~13us to
    half-clock windows); the drain-time junk heartbeat measured WORSE
    (crowds out phase-C scores) and is off by default.
  - b() transpose halves go to SEPARATE psum banks (concurrent row-group
    transpose streams into one bank = HW write-port collision -> NC
    error).
"""

import collections
import os
import numpy as np
import ml_dtypes

B, T, C, H = 8, 2048, 1024, 64
NCORES = 8
CHUNK = 512
NCHUNK = T // CHUNK  # 4
NSB = T // 128       # 16 s-blocks
NCB = C // 128       # 8 c-blocks
VHALF = CHUNK // 2   # 256

bf16 = ml_dtypes.bfloat16

JUNK_ON = os.environ.get("K_JUNK", "0") == "1"
MAXPHASE = int(os.environ.get("K_MAXPHASE", "3"))

SHUF_MASK = [i ^ 1 for i in range(32)]
PI = float(np.pi)
TWO_PI = float(2 * np.pi)


# ---------------------------------------------------------------- host consts
def _build_consts():
    half = H // 2
    inv_freq = (1.0 / (10000.0 ** (np.arange(half, dtype=np.float32) / half))).astype(
        np.float32
    )
    # unique-row rope table: 32 sin rows + 32 cos rows (the full [128, T]
    # tables repeat each frequency row 4x with signs); expanded on-chip by
    # a PE selector matmul. sctab rows 0:32 = sin(t*f_j), 32:64 = cos.
    t = np.arange(T, dtype=np.float64)
    ang = t[None, :] * inv_freq.astype(np.float64)[:, None]  # (32, T)
    sctab = np.concatenate([np.sin(ang), np.cos(ang)], axis=0).astype(bf16)

    # selectors [64, 2, 128]: out_row h of table k = sum_j sel[j, k, h] *
    # sctab[j]; sin rows get the rope rotation sign (-1 on even rows),
    # cos rows are plain repeats
    sel = np.zeros((64, 2, 128), np.float32)
    for h in range(128):
        hp = h % 64
        j = hp // 2
        sel[j, 0, h] = 1.0          # sinsin
        sel[32 + j, 1, h] = 1.0     # coscos
    sel = sel.astype(bf16)

    sl = np.arange(128)
    trimask = (sl[:, None] <= sl[None, :]).astype(bf16)  # (128, 128)

    identb2 = np.concatenate([np.eye(H), np.eye(H)], axis=0).astype(bf16)  # (128, 64)

    # rot matrix (PE): rot = R2.T @ qk with rot[2i] = -qk[2i+1], rot[2i+1] = qk[2i]
    Rm = np.zeros((H, H), np.float32)
    for i in range(H // 2):
        Rm[2 * i, 2 * i + 1] = -1.0
        Rm[2 * i + 1, 2 * i] = 1.0
    r2 = np.zeros((128, 128), np.float32)
    r2[0:H, 0:H] = Rm.T
    r2[H:128, H:128] = Rm.T
    r2 = r2.astype(bf16)

    # half-swap permutation: swapk = swapm.T @ qkrope puts k' at rows 0:64
    # and q' at rows 64:128 (out[m] = qkrope[(m+64)%128])
    swapm = np.zeros((128, 128), np.float32)
    swapm[(np.arange(128) + H) % 128, np.arange(128)] = 1.0
    swapm = swapm.astype(bf16)

    return sctab, sel, trimask, identb2, swapm, r2


# ---------------------------------------------------------------- bass program
def _build_bass():
    import concourse.mybir as mybir
    import concourse.tile as tile
    from concourse import bacc
    from concourse.bass import ts

    BF = mybir.dt.bfloat16
    F32 = mybir.dt.float32
    Exp = mybir.ActivationFunctionType.Exp
    Sin = mybir.ActivationFunctionType.Sin
    Mod = mybir.AluOpType.mod
    Mult = mybir.AluOpType.mult
    Add = mybir.AluOpType.add
    Min = mybir.AluOpType.min

    nc = bacc.Bacc(
        "TRN2",
        target_bir_lowering=False,
        debug=False,
        enable_asserts=False,
        num_devices=NCORES,
    )

    # xT prepacked on host: [chunk, half, 128, 4, 512] -> each (chunk,
    # half) DMA piece is a fully contiguous 512KB burst (few pieces: DMA
    # completion receipts cost ~2us each and pace the ramp)
    xT_d = nc.dram_tensor(
        "xTp", [NCHUNK, 2, 128, NCB // 2, CHUNK], BF, kind="ExternalInput"
    )
    wqk_d = nc.dram_tensor("wqkp", [128, NCB, 128], BF, kind="ExternalInput")
    wv_d = nc.dram_tensor("wvp", [128, NCB, H], BF, kind="ExternalInput")
    sctab_d = nc.dram_tensor("sctab", [H, T], BF, kind="ExternalInput")
    sel_d = nc.dram_tensor("sel", [H, 2, 128], BF, kind="ExternalInput")
    trimask_d = nc.dram_tensor("trimask", [128, 128], BF, kind="ExternalInput")
    identb_d = nc.dram_tensor("identb2", [128, H], BF, kind="ExternalInput")
    swapm_d = nc.dram_tensor("swapm", [128, 128], BF, kind="ExternalInput")
    r2_d = nc.dram_tensor("r2", [128, 128], BF, kind="ExternalInput")
    # raw [num(64); den(1)] per chunk; host divides
    out_d = nc.dram_tensor("out", [NCHUNK, H + 1, CHUNK], BF, kind="ExternalOutput")

    with tile.TileContext(nc) as tc:
        with (
            tc.tile_pool(name="persist", bufs=1) as persist,
            tc.tile_pool(name="work", bufs=3) as work,
            tc.tile_pool(name="pexpp", bufs=7) as pexpp,
            tc.tile_pool(name="ps_scratch", bufs=2, space="PSUM") as ps_scratch,
            tc.tile_pool(name="ps_sc", bufs=2, space="PSUM") as ps_sc,
            tc.tile_pool(name="ps_out", bufs=2, space="PSUM") as ps_out,
        ):
            # ---- persistent SBUF tensors
            wqk_sb = persist.tile([128, NCB, 128], BF)
            wv_sb = persist.tile([128, NCB, H], BF)
            coscos_sb = persist.tile([128, T], BF)
            sinsin_sb = persist.tile([128, T], BF)
            trimask_sb = persist.tile([128, 128], BF)
            identb_sb = persist.tile([128, H], BF)
            swapm_sb = persist.tile([128, 128], BF)
            r2_sb = persist.tile([128, 128], BF)
            sctab_sb = persist.tile([H, T], BF)
            sel_sb = persist.tile([H, 2, 128], BF)
            xT_sb = persist.tile([128, NCHUNK, NCB, CHUNK], BF)
            qkrope = persist.tile([128, T], BF)   # q' rows 0:64, k' rows 64:128
            swapk = persist.tile([128, T], BF)    # k' rows 0:64, q' rows 64:128
            vT_sb = persist.tile([128, NCHUNK, VHALF], BF)  # per-chunk col-halves
            vnat = persist.tile([128, NSB, H + 1], BF)

            # ---- sync HWDGE queue: ordered by need time; the rope table
            # first (its psum eviction gates chunk 0's projection slot)
            nc.sync.dma_start(out=sctab_sb[:], in_=sctab_d.ap())
            nc.sync.dma_start(out=sel_sb[:], in_=sel_d.ap())
            nc.sync.dma_start(out=wqk_sb[:], in_=wqk_d.ap())
            for i in range(NCHUNK):
                for h in range(2):
                    nc.sync.dma_start(
                        out=xT_sb[:, i, 4 * h : 4 * h + 4], in_=xT_d.ap()[i, h]
                    )
                if i == 0:
                    nc.sync.dma_start(out=wv_sb[:], in_=wv_d.ap())
            # ---- gpsimd (SWDGE): small consts on the other ring
            nc.gpsimd.dma_start(out=identb_sb[:], in_=identb_d.ap())
            nc.gpsimd.dma_start(out=swapm_sb[:], in_=swapm_d.ap())
            nc.gpsimd.dma_start(out=r2_sb[:], in_=r2_d.ap())
            nc.gpsimd.dma_start(out=trimask_sb[:], in_=trimask_d.ap())

            # PE warmup: junk matmuls so the HAM clock-gate starts opening
            # while the first DMAs land
            zwarm = persist.tile([128, CHUNK], BF)
            nc.vector.memset(zwarm[:], 0.0)
            nc.vector.memset(vnat[:], 1.0)  # ones col (64); cols 0:64 overwritten
            warm_ps = ps_sc.tile([128, 2, CHUNK], F32, tag="sc", name="warm")
            for w in range(3):
                nc.tensor.matmul(
                    warm_ps[:, 0, :],
                    zwarm[:, 0:128],
                    zwarm[:],
                    start=(w == 0),
                    stop=(w == 2),
                )

            # ---- expand the [64, T] unique-row table to the full
            # [128, T] sinsin/coscos tables via PE selector matmuls (exact
            # bf16 passthrough); per chunk so chunk 0's rope isn't queued
            # behind later chunks' table-eviction copies on the DVE
            def emit_extab(i, on_act=False):
                tsl0 = ts(i, CHUNK)
                for tab_sb, k in ((sinsin_sb, 0), (coscos_sb, 1)):
                    e_ps = ps_scratch.tile(
                        [128, CHUNK], F32, tag="scr", name=f"ex{i}_{k}"
                    )
                    nc.tensor.matmul(
                        e_ps[:], sel_sb[:, k, :], sctab_sb[:, tsl0],
                        start=True, stop=True,
                    )
                    if on_act:
                        # ACT is idle during the ramp; keeps the eviction off
                        # the DVE critical path and un-gates the scr slot
                        nc.scalar.copy(out=tab_sb[:, tsl0], in_=e_ps[:])
                    else:
                        nc.vector.tensor_copy(out=tab_sb[:, tsl0], in_=e_ps[:])

            # ---------------- emission units
            qk_tiles = {}

            def emit_qk_alloc(i):
                qk_tiles[i] = ps_scratch.tile(
                    [128, CHUNK], F32, tag="scr", name=f"qk{i}"
                )

            def emit_qk(i, c2):
                # 2 c-blocks of the qk projection accumulation for chunk i
                qk_ps = qk_tiles[i]
                for c in (c2, c2 + 1):
                    nc.tensor.matmul(
                        qk_ps[:],
                        wqk_sb[:, c, :],
                        xT_sb[:, i, c, :],
                        start=(c == 0),
                        stop=(c == NCB - 1),
                        skip_group_check=True,
                    )

            qkS_tiles = {}
            rot_tiles = {}

            def emit_cast(i):
                qkS = work.tile([128, CHUNK], BF, tag="qkS", name=f"qkS{i}")
                qkS_tiles[i] = qkS
                nc.vector.tensor_copy(out=qkS[:], in_=qk_tiles[i][:])

            def emit_rot(i):
                rot_ps = ps_scratch.tile(
                    [128, CHUNK], F32, tag="scr", name=f"rot{i}"
                )
                rot_tiles[i] = rot_ps
                nc.tensor.matmul(
                    rot_ps[:], r2_sb[:], qkS_tiles[i][:], start=True, stop=True
                )

            def emit_ropemul(i):
                tsl = ts(i, CHUNK)
                tmp1 = work.tile([128, CHUNK], BF, tag="tmp1", name=f"t1_{i}")
                nc.vector.tensor_mul(tmp1[:], qkS_tiles[i][:], coscos_sb[:, tsl])
                tmp2 = work.tile([128, CHUNK], BF, tag="tmp2", name=f"t2_{i}")
                nc.vector.tensor_mul(tmp2[:], rot_tiles[i][:], sinsin_sb[:, tsl])
                nc.vector.tensor_add(qkrope[:, tsl], tmp1[:], tmp2[:])

            def emit_swap(i):
                # half-swapped copy for the two row-tiled score streams via a
                # PE permutation matmul (an SBUF-SBUF DMA here has ~5us
                # completion latency under HBM load and gates the scores)
                tsl = ts(i, CHUNK)
                swap_ps = ps_scratch.tile(
                    [128, CHUNK], F32, tag="scr", name=f"swp{i}"
                )
                nc.tensor.matmul(
                    swap_ps[:], swapm_sb[:], qkrope[:, tsl], start=True, stop=True
                )
                nc.vector.tensor_copy(out=swapk[:, tsl], in_=swap_ps[:])

            v_tiles = {}

            def emit_v_alloc(i):
                v_tiles[i] = ps_scratch.tile(
                    [128, VHALF], F32, tag="scr", name=f"v{i}"
                )

            def emit_v(i, c2):
                # 2 c-blocks of chunk i's v-projection, col-tiled over the
                # chunk's two 256-halves so both PE column groups stream
                # concurrently
                v_ps = v_tiles[i]
                for c in (c2, c2 + 1):
                    nc.tensor.matmul(
                        v_ps[0:H, :],
                        wv_sb[:, c, :],
                        xT_sb[:, i, c, 0:VHALF],
                        start=(c == 0),
                        stop=(c == NCB - 1),
                        skip_group_check=True,
                    )
                    nc.tensor.matmul(
                        v_ps[H:128, :],
                        wv_sb[:, c, :],
                        xT_sb[:, i, c, VHALF:CHUNK],
                        start=(c == 0),
                        stop=(c == NCB - 1),
                        skip_group_check=True,
                    )
                if c2 == NCB - 2:
                    nc.vector.tensor_copy(out=vT_sb[:, i, :], in_=v_ps[:])

            def emit_b(i):
                # transpose chunk i's 4 s-blocks to natural layout; one psum
                # tile per row-group half: the two transpose streams run
                # concurrently and must drain into DIFFERENT banks
                for half_ in range(2):
                    vn_ps = ps_scratch.tile(
                        [128, 2, H], BF, tag="scr", name=f"vn{i}_{half_}"
                    )
                    base = H * half_
                    for j in range(2):
                        nc.tensor.transpose(
                            vn_ps[:, j, :],
                            vT_sb[base : base + H, i, ts(j, 128)],
                            identb_sb[base : base + H, :],
                        )
                    first = 4 * i + 2 * half_
                    nc.vector.tensor_copy(
                        out=vnat[:, first : first + 2, 0:H], in_=vn_ps[:]
                    )

            out_tiles = {}

            def emit_evict(i):
                out_ps = out_tiles[i]
                outS = work.tile([H + 1, CHUNK], BF, tag="outS", name=f"oS{i}")
                nc.vector.tensor_copy(out=outS[:], in_=out_ps[:])
                nc.gpsimd.dma_start(out=out_d.ap()[i], in_=outS[:])

            # ---------------- filler queue (cost_ns, key, fn)
            fillers = collections.deque()
            done_keys = set()
            junk_budget = [24 if JUNK_ON else 0]

            def drain_one():
                cost, key, fn = fillers.popleft()
                fn()
                if key is not None:
                    done_keys.add(key)
                return cost

            def drain(budget):
                while fillers and budget > 0:
                    budget -= drain_one()
                # fillers dry: keep the PE dense so HAM holds K=8/8
                while budget > 0 and junk_budget[0] > 0:
                    junk_ps = ps_scratch.tile(
                        [128, CHUNK], F32, tag="scr",
                        name=f"junk{junk_budget[0]}",
                    )
                    nc.tensor.matmul(
                        junk_ps[:], zwarm[:, 0:128], zwarm[:],
                        start=True, stop=True, skip_group_check=True,
                    )
                    junk_budget[0] -= 1
                    budget -= 216

            def drain_until(key):
                if key in done_keys:
                    return
                while fillers:
                    cost, k, fn = fillers[0]
                    drain_one()
                    if k == key:
                        return

            def queue_A(i):
                def qk_unit(c2, first):
                    def fn():
                        if first:
                            emit_qk_alloc(i)
                        emit_qk(i, c2)
                    return fn

                for c2 in range(0, NCB, 2):
                    fillers.append((432, None, qk_unit(c2, c2 == 0)))
                fillers.append((100, None, lambda: emit_cast(i)))
                fillers.append((216, None, lambda: emit_rot(i)))
                fillers.append((100, None, lambda: emit_ropemul(i)))
                fillers.append((300, ("rotrope", i), lambda: emit_swap(i)))

            def queue_V(i):
                def v_unit(c2, first):
                    def fn():
                        if first:
                            emit_v_alloc(i)
                        emit_v(i, c2)
                    return fn

                for c2 in range(0, NCB, 2):
                    fillers.append((440, None, v_unit(c2, c2 == 0)))
                fillers.append((300, ("b", i), lambda: emit_b(i)))

            # ---------------- phase C
            def phase_c(i, diag_pos=None, budget=500, entry_budget=0):
                drain_until(("rotrope", i))
                # pop the NEXT chunk's projection+rope fillers now: their
                # DVE chain then queues ahead of this phase's trimask/evict
                # work and completes during our exps instead of gating the
                # next phase's first scores
                drain(entry_budget)
                bkey = ("b", i)
                out_ps = ps_out.tile([H + 1, CHUNK], F32, tag="out", name=f"o{i}")
                out_tiles[i] = out_ps
                started = [False]

                punits = [("pair", p) for p in range(2 * i)]
                dunits = [("diag", 0), ("diag", 1)]
                if diag_pos is None:
                    diag_pos = len(punits)
                units = punits[:diag_pos] + dunits + punits[diag_pos:]
                staged = []

                def emit_scores(u):
                    kind, idx = u
                    sc2 = ps_sc.tile(
                        [128, 2, CHUNK], F32, tag="sc", name=f"s{i}_{kind}{idx}"
                    )
                    if kind == "pair":
                        sb = 2 * idx
                        nc.tensor.matmul(
                            sc2[:, 0, :],
                            swapk[0:H, ts(sb, 128)],
                            qkrope[0:H, ts(i, CHUNK)],
                            start=True,
                            stop=True,
                        )
                        nc.tensor.matmul(
                            sc2[:, 1, :],
                            qkrope[H:128, ts(sb + 1, 128)],
                            swapk[H:128, ts(i, CHUNK)],
                            start=True,
                            stop=True,
                        )
                    else:
                        j0 = 2 * idx
                        lo0 = 128 * j0
                        nc.tensor.matmul(
                            sc2[:, 0, lo0:CHUNK],
                            swapk[0:H, ts(4 * i + j0, 128)],
                            qkrope[0:H, i * CHUNK + lo0 : (i + 1) * CHUNK],
                            start=True,
                            stop=True,
                        )
                        # stream B starts at lo0 too: the extra cols ride in
                        # stream A's concurrency shadow and initialize the
                        # region the merged exp reads
                        nc.tensor.matmul(
                            sc2[:, 1, lo0:CHUNK],
                            qkrope[H:128, ts(4 * i + j0 + 1, 128)],
                            swapk[H:128, i * CHUNK + lo0 : (i + 1) * CHUNK],
                            start=True,
                            stop=True,
                        )
                    staged.append((kind, idx, sc2))

                def emit_exp_num(stage, last_unit):
                    kind, idx, sc2 = stage
                    pexp2 = pexpp.tile(
                        [128, 2, CHUNK], BF, tag="pexp", name=f"p{i}_{kind}{idx}"
                    )
                    if kind == "pair":
                        nc.scalar.activation(
                            out=pexp2[:], in_=sc2[:], func=Exp, scale=0.125
                        )
                        for h_ in range(2):
                            sb = 2 * idx + h_
                            st = not started[0]
                            started[0] = True
                            nc.tensor.matmul(
                                out_ps[:],
                                vnat[:, sb, :],
                                pexp2[:, h_, :],
                                start=st,
                                stop=(last_unit and h_ == 1),
                                skip_group_check=True,
                            )
                    else:
                        j0 = 2 * idx
                        lo0 = 128 * j0
                        nc.scalar.activation(
                            out=pexp2[:, :, lo0:CHUNK],
                            in_=sc2[:, :, lo0:CHUNK],
                            func=Exp,
                            scale=0.125,
                        )
                        # trimask corners on DVE (off PE critical path)
                        for h_ in range(2):
                            lo = 128 * (j0 + h_)
                            nc.vector.tensor_mul(
                                pexp2[:, h_, lo : lo + 128],
                                pexp2[:, h_, lo : lo + 128],
                                trimask_sb[:],
                            )
                        halves = [0, 1]
                        if not started[0]:
                            # a start=True matmul resets the WHOLE psum tile,
                            # so the chunk's first AV must be one full-width
                            # start=True instruction (corner already masked)
                            assert j0 == 0
                            started[0] = True
                            nc.tensor.matmul(
                                out_ps[:],
                                vnat[:, 4 * i, :],
                                pexp2[:, 0, :],
                                start=True,
                                stop=False,
                                skip_group_check=True,
                            )
                            halves = [1]
                        # unmasked AV tails fire straight after exp
                        for h_ in halves:
                            sb = 4 * i + j0 + h_
                            lo = 128 * (j0 + h_)
                            if lo + 128 < CHUNK:
                                nc.tensor.matmul(
                                    out_ps[:, lo + 128 : CHUNK],
                                    vnat[:, sb, :],
                                    pexp2[:, h_, lo + 128 : CHUNK],
                                    start=False,
                                    stop=False,
                                    skip_group_check=True,
                                )
                        # masked corners after the DVE multiplies
                        for h_ in halves:
                            sb = 4 * i + j0 + h_
                            lo = 128 * (j0 + h_)
                            nc.tensor.matmul(
                                out_ps[:, lo : lo + 128],
                                vnat[:, sb, :],
                                pexp2[:, h_, lo : lo + 128],
                                start=False,
                                stop=(last_unit and h_ == 1),
                                skip_group_check=True,
                            )

                first_diag = diag_pos
                for n, u in enumerate(units):
                    if n == first_diag:
                        drain_until(bkey)
                    emit_scores(u)
                    if n > 0:
                        emit_exp_num(staged.pop(0), False)
                    drain(budget)
                emit_exp_num(staged.pop(0), True)

            # ---------------- top-level schedule: per-chunk pipeline with
            # A/V/b of later chunks as fillers inside earlier C phases
            # junk bridge: keep the PE busy until the rope table + x-stream
            # land (a >3.4us idle window re-throttles HAM)
            for w in range(12):
                nc.tensor.matmul(
                    warm_ps[:, 1, :], zwarm[:, 0:128], zwarm[:],
                    start=(w == 0), stop=(w == 11), skip_group_check=True,
                )
            emit_extab(0, on_act=True)
            # chunk-0 chain, ordered so no PE instruction waits on a DVE
            # result that is produced behind it in the queue
            emit_qk_alloc(0)
            for c2 in range(0, NCB, 2):
                emit_qk(0, c2)
            emit_cast(0)
            emit_v_alloc(0)
            for c2 in range(0, NCB, 2):
                emit_v(0, c2)
            emit_rot(0)
            emit_ropemul(0)
            emit_b(0)
            done_keys.add(("b", 0))
            emit_swap(0)
            done_keys.add(("rotrope", 0))
            emit_extab(1, on_act=True)
            # A(1) qk matmuls directly: they fill the PE idle window while
            # the chunk-0 rope chain runs on the DVE (the scores that follow
            # them in the queue wait on that chain anyway)
            emit_qk_alloc(1)
            for c2 in range(0, NCB, 2):
                emit_qk(1, c2)

            fillers.append((100, None, lambda: emit_cast(1)))
            fillers.append((216, None, lambda: emit_rot(1)))
            fillers.append((100, None, lambda: emit_ropemul(1)))
            fillers.append((300, ("rotrope", 1), lambda: emit_swap(1)))
            queue_V(1)
            fillers.append((450, None, lambda: emit_extab(2)))
            queue_A(2)
            queue_V(2)
            fillers.append((450, None, lambda: emit_extab(3)))
            queue_A(3)
            queue_V(3)

            phase_c(0, budget=3000)
            emit_evict(0)
            if MAXPHASE >= 1:
                phase_c(1, budget=2200)
                emit_evict(1)
            if MAXPHASE >= 2:
                phase_c(2, budget=1100)
                emit_evict(2)
            if MAXPHASE >= 3:
                phase_c(3, diag_pos=2, budget=500)
                emit_evict(3)

    nc.compile()
    return nc


_NC_CACHE = None


def _get_nc():
    global _NC_CACHE
    if _NC_CACHE is None:
        _NC_CACHE = _build_bass()
    return _NC_CACHE


def make_in_maps(x, Wq, Wk, Wv):
    """Host-side prep: shard over batch + precompute constants."""
    sctab, sel, trimask, identb2, swapm, r2 = _build_consts()
    wqk = np.concatenate([Wq, Wk], axis=1).astype(bf16)  # (C, 128)
    wv = Wv.astype(bf16)
    wqkp = np.ascontiguousarray(wqk.reshape(NCB, 128, 128).transpose(1, 0, 2))
    wvp = np.ascontiguousarray(wv.reshape(NCB, 128, H).transpose(1, 0, 2))
    in_maps = []
    for b in range(B):
        xT = x[b].T.astype(bf16)  # (C, T)
        # [chunk, half, 128, 4, 512]: contiguous 512KB per (chunk, half)
        xTp = np.ascontiguousarray(
            xT.reshape(2, 4, 128, NCHUNK, CHUNK).transpose(3, 0, 2, 1, 4)
        )
        in_maps.append(
            {
                "xTp": xTp,
                "wqkp": wqkp,
                "wvp": wvp,
                "sctab": sctab,
                "sel": sel,
                "trimask": trimask,
                "identb2": identb2,
                "swapm": swapm,
                "r2": r2,
            }
        )
    return in_maps


def postprocess(raw):
    """[NCHUNK, 65, CHUNK] num/den -> (T, H) normalized output."""
    raw = np.asarray(raw, dtype=np.float32)
    num = raw[:, 0:H, :]                  # (4, 64, 512)
    den = raw[:, H, :]                    # (4, 512)
    out = num / den[:, None, :]
    return np.ascontiguousarray(out.transpose(0, 2, 1).reshape(T, H))


def kernel(x, Wq, Wk, Wv):
    from concourse.bass_utils import run_bass_kernel_spmd

    x = np.asarray(x, dtype=np.float32)
    Wq = np.asarray(Wq, dtype=np.float32)
    Wk = np.asarray(Wk, dtype=np.float32)
    Wv = np.asarray(Wv, dtype=np.float32)

    nc = _get_nc()
    in_maps = make_in_maps(x, Wq, Wk, Wv)
    res = run_bass_kernel_spmd(nc, in_maps, core_ids=list(range(NCORES)))
    out = np.stack([postprocess(r["out"]) for r in res.results])  # (B, T, H)
    return np.ascontiguousarray(out.astype(np.float32))
